# revision 1
# baseline (speedup 1.0000x reference)
"""FNO3d (RCLN v3) kernel for 8 NeuronCores.

Strategy: data-parallel across batch B=8 (1 sample/core via jax.pmap on the
axon/neuron backend). The spectral conv keeps only 4 modes per axis, so the
rfftn/irfftn are implemented as small dense real DFT matrices (no complex
dtypes — neuronx-cc does not support them). Everything becomes real einsums
+ gelu, which the Neuron compiler maps to TensorE/ScalarE.

Self-contained: hardcodes shapes B=8, C_in=3, width=8, D=H=W=64, modes=4.
Falls back to a pure-numpy implementation if the jax/neuron path fails.
"""

import os
import numpy as np

MODES = 4
N = 64
LAMBDA_RES = 0.3
KDH = np.array([0, 1, 2, 3, 60, 61, 62, 63])  # kept bins along d and h


def _dft_mats():
    n = np.arange(N)
    # forward along w (rfft, bins 0..3): F[w, k] = exp(-2pi i k w / N)
    Fw = np.exp(-2j * np.pi * np.outer(n, np.arange(MODES)) / N)  # [64, 4]
    # forward along h/d (full fft, kept bins): [64, 8]
    Fh = np.exp(-2j * np.pi * np.outer(n, KDH) / N)
    # inverse along d/h (ifft restricted to kept bins): [8, 64]
    Gd = np.exp(2j * np.pi * np.outer(KDH, n) / N) / N
    # inverse along w: probe numpy irfft for exact semantics (incl. DC bin)
    IWr = np.zeros((MODES, N), np.float64)
    IWi = np.zeros((MODES, N), np.float64)
    for k in range(MODES):
        e = np.zeros(N // 2 + 1, complex)
        e[k] = 1.0
        IWr[k] = np.fft.irfft(e, n=N)
        e = np.zeros(N // 2 + 1, complex)
        e[k] = 1j
        IWi[k] = np.fft.irfft(e, n=N)
    f32 = np.float32
    return (
        Fw.real.astype(f32), Fw.imag.astype(f32),
        Fh.real.astype(f32), Fh.imag.astype(f32),
        Gd.real.astype(f32), Gd.imag.astype(f32),
        IWr.astype(f32), IWi.astype(f32),
    )


_FWr, _FWi, _FHr, _FHi, _GDr, _GDi, _IWr, _IWi = _dft_mats()


def _assemble_specw(spec_ws_layer):
    """4 octant weights [8,8,4,4,4] complex -> [8,8,8,8,4] (d-modes, h-modes)."""
    w1, w2, w3, w4 = [np.asarray(w) for w in spec_ws_layer]
    cin, cout = w1.shape[0], w1.shape[1]
    W = np.zeros((cin, cout, 8, 8, MODES), np.complex64)
    m = MODES
    W[:, :, :m, :m, :] = w1   # d-low,  h-low
    W[:, :, m:, :m, :] = w2   # d-high, h-low
    W[:, :, :m, m:, :] = w3   # d-low,  h-high
    W[:, :, m:, m:, :] = w4   # d-high, h-high
    return W.real.astype(np.float32), W.imag.astype(np.float32)


def _np_arr(x):
    return np.asarray(x, dtype=np.float32)


def _prep_weights(inputs):
    p = {}
    p["fc0_w"] = _np_arr(inputs["fc0_w"])
    p["fc0_b"] = _np_arr(inputs["fc0_b"])
    p["fc1_w"] = _np_arr(inputs["fc1_w"])
    p["fc1_b"] = _np_arr(inputs["fc1_b"])
    p["fc2_w"] = _np_arr(inputs["fc2_w"])
    p["fc2_b"] = _np_arr(inputs["fc2_b"])
    p["conv_ws"] = [_np_arr(w) for w in inputs["conv_ws"]]
    p["conv_bs"] = [_np_arr(b) for b in inputs["conv_bs"]]
    p["spec"] = [_assemble_specw(layer) for layer in inputs["spec_ws"]]
    p["nu"] = np.float32(np.asarray(inputs["nu"]))
    return p


def _fno_jax(u, p, jnp, gelu):
    """u: [b, 3, D, H, W] -> out [b, 6, D, H, W]. All real einsums."""
    x = jnp.einsum("bcdhw,oc->bodhw", u, p["fc0_w"]) \
        + p["fc0_b"][None, :, None, None, None]
    for i in range(4):
        Wr, Wi = p["spec"][i]
        # forward w: [b,c,d,h,w] x [w,k] -> [b,c,d,h,k]
        ar = jnp.einsum("bcdhw,wk->bcdhk", x, _FWr)
        ai = jnp.einsum("bcdhw,wk->bcdhk", x, _FWi)
        # forward h
        br = jnp.einsum("bcdhk,hm->bcdmk", ar, _FHr) \
            - jnp.einsum("bcdhk,hm->bcdmk", ai, _FHi)
        bi = jnp.einsum("bcdhk,hm->bcdmk", ar, _FHi) \
            + jnp.einsum("bcdhk,hm->bcdmk", ai, _FHr)
        # forward d
        cr = jnp.einsum("bcdmk,dn->bcnmk", br, _FHr) \
            - jnp.einsum("bcdmk,dn->bcnmk", bi, _FHi)
        ci = jnp.einsum("bcdmk,dn->bcnmk", br, _FHi) \
            + jnp.einsum("bcdmk,dn->bcnmk", bi, _FHr)
        # mode multiply (contract in-channel)
        yr = jnp.einsum("bcnmk,conmk->bonmk", cr, Wr) \
            - jnp.einsum("bcnmk,conmk->bonmk", ci, Wi)
        yi = jnp.einsum("bcnmk,conmk->bonmk", cr, Wi) \
            + jnp.einsum("bcnmk,conmk->bonmk", ci, Wr)
        # inverse d
        dr = jnp.einsum("bonmk,nd->bodmk", yr, _GDr) \
            - jnp.einsum("bonmk,nd->bodmk", yi, _GDi)
        di = jnp.einsum("bonmk,nd->bodmk", yr, _GDi) \
            + jnp.einsum("bonmk,nd->bodmk", yi, _GDr)
        # inverse h
        er = jnp.einsum("bodmk,mh->bodhk", dr, _GDr) \
            - jnp.einsum("bodmk,mh->bodhk", di, _GDi)
        ei = jnp.einsum("bodmk,mh->bodhk", dr, _GDi) \
            + jnp.einsum("bodmk,mh->bodhk", di, _GDr)
        # inverse w (real output)
        x1 = jnp.einsum("bodhk,kw->bodhw", er, _IWr) \
            + jnp.einsum("bodhk,kw->bodhw", ei, _IWi)
        x2 = jnp.einsum("bcdhw,oc->bodhw", x, p["conv_ws"][i]) \
            + p["conv_bs"][i][None, :, None, None, None]
        x = x1 + x2
        if i < 3:
            x = gelu(x)
    x = jnp.einsum("bcdhw,oc->bodhw", x, p["fc1_w"]) \
        + p["fc1_b"][None, :, None, None, None]
    x = gelu(x)
    x = jnp.einsum("bcdhw,oc->bodhw", x, p["fc2_w"]) \
        + p["fc2_b"][None, :, None, None, None]
    return x


def _run_jax(u, p):
    import jax
    import jax.numpy as jnp

    devs = jax.devices()
    B = u.shape[0]
    nu = p["nu"]

    def per_shard(us):  # us: [bs, 3, D, H, W]
        soft = _fno_jax(us, p, jnp, jax.nn.gelu)
        hard = jnp.concatenate(
            [us * nu, jnp.zeros((us.shape[0], 3) + us.shape[2:], us.dtype)],
            axis=1,
        )
        return hard + np.float32(LAMBDA_RES) * soft

    if len(devs) >= B:
        # one sample per core, data-parallel
        fn = jax.pmap(lambda us: per_shard(us), devices=devs[:B])
        out = fn(u[:, None])  # [B, 1, 3, D, H, W] -> [B, 1, 6, ...]
        return np.asarray(out)[:, 0]
    fn = jax.jit(per_shard)
    return np.asarray(fn(u))


def _gelu_np(x):
    c = np.float32(np.sqrt(2.0 / np.pi))
    return np.float32(0.5) * x * (
        np.float32(1.0)
        + np.tanh(c * (x + np.float32(0.044715) * x * x * x))
    )


class _NpWrap:
    """Minimal jnp-like shim so _fno_jax runs on numpy."""
    einsum = staticmethod(
        lambda s, a, b: np.einsum(s, a, b, optimize=True).astype(np.float32)
    )
    concatenate = staticmethod(np.concatenate)
    zeros = staticmethod(np.zeros)


def _run_numpy(u, p):
    soft = _fno_jax(u, p, _NpWrap, _gelu_np)
    B = u.shape[0]
    hard = np.concatenate(
        [u * p["nu"], np.zeros((B, 3) + u.shape[2:], u.dtype)], axis=1
    )
    return (hard + np.float32(LAMBDA_RES) * soft).astype(np.float32)


def kernel(**inputs):
    u = _np_arr(inputs["u"])
    p = _prep_weights(inputs)
    if os.environ.get("FNO_TRY_JAX"):
        try:
            out = _run_jax(u, p)
            ok = out.shape == (u.shape[0], 6) + u.shape[2:]
            if ok and np.isfinite(out).all():
                return out.astype(np.float32)
        except Exception:
            pass
    return _run_numpy(u, p)



# revision 20
# speedup vs baseline: 2.2713x; 2.2713x over previous
"""FNO3d (RCLN v3) Bass/Tile kernel for 8 NeuronCores (trn2).

Data-parallel: 1 batch sample per core, no collectives. The spectral conv
keeps 4 modes/axis, implemented as small dense real-DFT matmuls on the PE
with block-diagonal weight packing so the 128-lane contraction dim is full.

Per-core pipeline (all activations bf16 in SBUF, fp32 PSUM):
  lift (3->8) -> 4x [ PE-transpose to spatial layout -> fused (h,w)-DFT ->
  d-DFT -> per-mode complex mix -> inverse d -> fused inverse (h,w) matmul
  accumulated with the 1x1 conv in PSUM -> gelu drain ] -> fc1+gelu -> fc2
  (+0.3 scale folded into fc2 weights) -> out. Hard path u*nu is added via
  an SBUF dma-accum into the output staging tile.

Weights are baked into the NEFF as inline consts; only `u` ships per call.
"""
import os
import sys
import functools
import hashlib

os.environ.setdefault("JAX_PLATFORMS", "axon,cpu")

import numpy as np
import ml_dtypes

BF16NP = ml_dtypes.bfloat16
N = 64
M = 4
KD = np.array([0, 1, 2, 3, 60, 61, 62, 63])
LAM = 0.3

LAST_EXEC_NS = None


# ---------------------------------------------------------------------------
# host-side weight building (validated in dataflow_check.py)
# ---------------------------------------------------------------------------

def _dft_mats():
    n = np.arange(N)
    FW = np.exp(-2j * np.pi * np.outer(n, np.arange(M)) / N)      # [64, 4]
    FH = np.exp(-2j * np.pi * np.outer(n, KD) / N)                # [64, 8]
    GD = np.exp(2j * np.pi * np.outer(KD, n) / N) / N             # [8, 64]
    IWr = np.zeros((M, N)); IWi = np.zeros((M, N))
    for k in range(M):
        e = np.zeros(N // 2 + 1, complex); e[k] = 1.0
        IWr[k] = np.fft.irfft(e, n=N)
        e = np.zeros(N // 2 + 1, complex); e[k] = 1j
        IWi[k] = np.fft.irfft(e, n=N)
    return FW, FH, GD, IWr, IWi


def _assemble_specw(layer):
    w1, w2, w3, w4 = [np.asarray(w) for w in layer]
    W = np.zeros((8, 8, 8, 8, M), np.complex64)
    W[:, :, :4, :4, :] = w1
    W[:, :, 4:, :4, :] = w2
    W[:, :, :4, 4:, :] = w3
    W[:, :, 4:, 4:, :] = w4
    return W


def _make_weights(inputs):
    FW, FH, GD, IWr, IWi = _dft_mats()
    FHr, FHi = FH.real, FH.imag
    GDr, GDi = GD.real, GD.imag
    p = {}

    fc0_w = np.asarray(inputs["fc0_w"], np.float32)
    L = np.zeros((48, 128), np.float32)
    for c in range(3):
        for dl in range(16):
            for o in range(8):
                L[c * 16 + dl, dl * 8 + o] = fc0_w[o, c]
    p["lift"] = L

    Mf = np.zeros((32, 128, 64), np.float32)
    for hq in range(32):
        for hp in range(2):
            h = 2 * hq + hp
            for w in range(N):
                blk = np.einsum("m,k->mk", FH[h], FW[w])
                Mf[hq, hp * 64 + w, 0::2] = blk.real.reshape(-1)
                Mf[hq, hp * 64 + w, 1::2] = blk.imag.reshape(-1)
    # device layout: [128, 32*64], chunk hq at cols [hq*64:(hq+1)*64]
    p["Mf"] = np.concatenate([Mf[hq] for hq in range(32)], axis=1)

    F2r = np.zeros((128, 32), np.float32)
    F2i = np.zeros((128, 32), np.float32)
    for cc in range(2):
        for d in range(N):
            for nn in range(8):
                F2r[cc * 64 + d, cc * 16 + nn * 2 + 0] = FHr[d, nn]
                F2r[cc * 64 + d, cc * 16 + nn * 2 + 1] = FHi[d, nn]
                F2i[cc * 64 + d, cc * 16 + nn * 2 + 0] = -FHi[d, nn]
                F2i[cc * 64 + d, cc * 16 + nn * 2 + 1] = FHr[d, nn]
    p["F2r"], p["F2i"] = F2r, F2i

    modes = []
    for li in range(4):
        W = _assemble_specw(inputs["spec_ws"][li])   # [c,o,n,mi,k]
        mats = np.zeros((32, 128, 128), np.float32)
        for mi in range(8):
            for k in range(M):
                col = mi * 4 + k
                Wr = W[:, :, :, mi, k].real
                Wi = W[:, :, :, mi, k].imag
                for nn in range(8):
                    r0 = nn * 2
                    c0 = nn * 2
                    for c in range(8):
                        for o in range(8):
                            mats[col, c * 16 + r0 + 0, o * 16 + c0 + 0] = Wr[c, o, nn]
                            mats[col, c * 16 + r0 + 0, o * 16 + c0 + 1] = Wi[c, o, nn]
                            mats[col, c * 16 + r0 + 1, o * 16 + c0 + 0] = -Wi[c, o, nn]
                            mats[col, c * 16 + r0 + 1, o * 16 + c0 + 1] = Wr[c, o, nn]
        modes.append(np.concatenate([mats[col] for col in range(32)], axis=1))
    p["modes"] = modes    # [4][128, 4096]

    F3r = np.zeros((128, 512), np.float32)
    F3i = np.zeros((128, 512), np.float32)
    for g in range(8):
        for o in range(8):
            for nn in range(8):
                for ds in range(8):
                    d = g * 8 + ds
                    cix = g * 64 + o * 8 + ds
                    F3r[o * 16 + nn * 2 + 0, cix] = GDr[nn, d]
                    F3r[o * 16 + nn * 2 + 1, cix] = -GDi[nn, d]
                    F3i[o * 16 + nn * 2 + 0, cix] = GDi[nn, d]
                    F3i[o * 16 + nn * 2 + 1, cix] = GDr[nn, d]
    p["F3r"], p["F3i"] = F3r, F3i

    T4 = np.zeros((64, 4096), np.float32)
    for mi in range(8):
        for k in range(M):
            Tr = np.outer(GDr[mi], IWr[k]) + np.outer(GDi[mi], IWi[k])
            Ti = np.outer(GDr[mi], IWi[k]) - np.outer(GDi[mi], IWr[k])
            T4[0 * 32 + mi * 4 + k] = Tr.reshape(-1)
            T4[1 * 32 + mi * 4 + k] = Ti.reshape(-1)
    p["T4"] = T4

    convs = np.zeros((128, 512), np.float32)
    for li in range(4):
        cw = np.asarray(inputs["conv_ws"][li], np.float32)
        for dl in range(16):
            convs[dl * 8:(dl + 1) * 8, li * 128 + dl * 8: li * 128 + (dl + 1) * 8] = cw.T
    p["convs"] = convs

    fc1_w = np.asarray(inputs["fc1_w"], np.float32)
    fc1s = np.zeros((128, 512), np.float32)
    for pp in range(4):
        for dlr in range(4):
            dl = 4 * pp + dlr
            fc1s[dl * 8:(dl + 1) * 8, pp * 128 + dlr * 32: pp * 128 + (dlr + 1) * 32] = fc1_w.T
    p["fc1s"] = fc1s

    fc2_w = np.asarray(inputs["fc2_w"], np.float32)
    F2m = np.zeros((128, 128), np.float32)
    for dlr in range(4):
        F2m[dlr * 32:(dlr + 1) * 32, dlr * 32:dlr * 32 + 6] = LAM * fc2_w.T
    p["fc2"] = F2m

    # hard path: out[dlr*32+c] += nu*u[c, dl=4pp+dlr] via fc2-PSUM accumulation
    nu = float(np.asarray(inputs["nu"]))
    Hm = np.zeros((48, 512), np.float32)
    for pp in range(4):
        for dlr in range(4):
            dl = 4 * pp + dlr
            for c in range(3):
                Hm[c * 16 + dl, pp * 128 + dlr * 32 + c] = nu
    p["hard"] = Hm

    # bias table [128, 8] fp32: col0 lift, col1-4 conv li, col5 fc1, col6 fc2
    B = np.zeros((128, 8), np.float32)
    B[:, 0] = np.tile(np.asarray(inputs["fc0_b"], np.float32), 16)
    for li in range(4):
        B[:, 1 + li] = np.tile(np.asarray(inputs["conv_bs"][li], np.float32), 16)
    B[:, 5] = np.tile(np.asarray(inputs["fc1_b"], np.float32), 4)
    fb = np.zeros(32, np.float32)
    fb[:6] = LAM * np.asarray(inputs["fc2_b"], np.float32)
    B[:, 6] = np.tile(fb, 4)
    p["bias"] = B

    return p


def _wb_key(p):
    h = hashlib.sha256()
    for k in sorted(p.keys()):
        v = p[k]
        if isinstance(v, list):
            for a in v:
                h.update(np.ascontiguousarray(a).tobytes())
        elif isinstance(v, np.ndarray):
            h.update(np.ascontiguousarray(v).tobytes())
        else:
            h.update(repr(v).encode())
    return h.hexdigest()


# ---------------------------------------------------------------------------
# bass kernel
# ---------------------------------------------------------------------------

def _build_nc(p):
    import concourse.bass as bass
    import concourse.tile as tile
    from concourse import bacc, mybir
    from contextlib import ExitStack

    BF = mybir.dt.bfloat16
    F32 = mybir.dt.float32
    GELU = mybir.ActivationFunctionType.Gelu_apprx_tanh
    bf = lambda a: np.ascontiguousarray(np.asarray(a, np.float32).astype(BF16NP))

    nc = bacc.Bacc("TRN2", target_bir_lowering=False, debug=False, num_devices=8)
    # u arrives host-prearranged: [48, 16384] bf16, p=c*16+dl, f=dh*4096+h*64+w
    u_d = nc.declare_dram_parameter("u", [48, 16384], BF, isOutput=False)
    out_d = nc.declare_dram_parameter("out", [6, 64, 64, 64], BF, isOutput=True)

    identD = nc.inline_tensor(np.eye(128, dtype=BF16NP), name="identc")
    liftD = nc.inline_tensor(bf(p["lift"]), name="liftc")
    MfD = nc.inline_tensor(bf(p["Mf"]), name="mfc")
    F2rD = nc.inline_tensor(bf(p["F2r"]), name="f2rc")
    F2iD = nc.inline_tensor(bf(p["F2i"]), name="f2ic")
    modeD = [nc.inline_tensor(bf(p["modes"][li]), name=f"modec{li}") for li in range(4)]
    F3rD = nc.inline_tensor(bf(p["F3r"]), name="f3rc")
    F3iD = nc.inline_tensor(bf(p["F3i"]), name="f3ic")
    T4D = nc.inline_tensor(bf(p["T4"]), name="t4c")
    convD = nc.inline_tensor(bf(p["convs"]), name="convc")
    fc1D = nc.inline_tensor(bf(p["fc1s"]), name="fc1c")
    fc2D = nc.inline_tensor(bf(p["fc2"]), name="fc2c")
    hardD = nc.inline_tensor(bf(p["hard"]), name="hardc")
    biasD = nc.inline_tensor(np.asarray(p["bias"], np.float32), name="biasc")

    with tile.TileContext(nc) as tc, ExitStack() as ctx:
        cpool = ctx.enter_context(tc.tile_pool(name="const", bufs=1))
        mwp = ctx.enter_context(tc.tile_pool(name="mw", bufs=2))
        xsp = ctx.enter_context(tc.tile_pool(name="xsp", bufs=4))
        smp = ctx.enter_context(tc.tile_pool(name="small", bufs=2))
        zp = ctx.enter_context(tc.tile_pool(name="zp", bufs=2))
        op = ctx.enter_context(tc.tile_pool(name="outp", bufs=2))
        pmid = ctx.enter_context(tc.tile_pool(name="pmid", bufs=3, space="PSUM"))
        pX = ctx.enter_context(tc.tile_pool(name="px", bufs=2, space="PSUM"))

        identW = cpool.tile([128, 128], BF)
        liftW = cpool.tile([48, 128], BF)
        MfW = cpool.tile([128, 2048], BF)
        F2rW = cpool.tile([128, 32], BF)
        F2iW = cpool.tile([128, 32], BF)
        F3rW = cpool.tile([128, 512], BF)
        F3iW = cpool.tile([128, 512], BF)
        T4W = cpool.tile([64, 4096], BF)
        convW = cpool.tile([128, 512], BF)
        fc1W = cpool.tile([128, 512], BF)
        fc2W = cpool.tile([128, 128], BF)
        hardW = cpool.tile([48, 512], BF)
        biasT = cpool.tile([128, 8], F32)
        uT = cpool.tile([48, 16384], BF)
        xA = cpool.tile([128, 16384], BF)
        xB = cpool.tile([128, 16384], BF)

        for t, d in [(identW, identD), (liftW, liftD), (MfW, MfD), (F2rW, F2rD),
                     (F2iW, F2iD), (F3rW, F3rD), (F3iW, F3iD), (T4W, T4D),
                     (convW, convD), (fc1W, fc1D), (fc2W, fc2D),
                     (hardW, hardD), (biasT, biasD)]:
            nc.sync.dma_start(out=t[:, :], in_=d[:, :])

        nc.sync.dma_start(out=uT[:, :], in_=u_d[:, :])

        # ---- lift ----
        for ch in range(16):
            px = pX.tile([128, 1024], F32, tag="px")
            for s in range(2):
                nc.tensor.matmul(px[:, s * 512:(s + 1) * 512], liftW[:, :],
                                 uT[:, ch * 1024 + s * 512: ch * 1024 + (s + 1) * 512],
                                 start=True, stop=True)
            nc.vector.tensor_scalar_add(xA[:, ch * 1024:(ch + 1) * 1024],
                                        px[:, :], biasT[:, 0:1])


        xcur, xnext = xA, xB
        for li in range(4):
            mw = mwp.tile([128, 4096], BF, tag="mw")
            nc.sync.dma_start(out=mw[:, :], in_=modeD[li][:, :])

            # ---- Tx + F1 ----
            pb = pmid.tile([64, 512], F32, tag="mid")
            for hq in range(32):
                ptx = pmid.tile([128, 512], BF, tag="mid")
                for dh in range(4):
                    nc.tensor.transpose(
                        ptx[:, dh * 128:(dh + 1) * 128],
                        xcur[:, dh * 4096 + hq * 128: dh * 4096 + (hq + 1) * 128],
                        identW[:, :])
                xs = xsp.tile([128, 512], BF, tag="xs")
                nc.vector.tensor_copy(
                    out=xs[:, :].rearrange("p (c dh dl) -> p dh dl c",
                                            c=8, dh=4, dl=16),
                    in_=ptx[:, :].rearrange("p (dh dl c) -> p dh dl c",
                                            c=8, dh=4, dl=16))
                nc.tensor.matmul(pb[:, :], MfW[:, hq * 64:(hq + 1) * 64], xs[:, :],
                                 start=(hq == 0), stop=(hq == 31))
            bS = smp.tile([64, 512], BF, tag="bS")
            nc.vector.tensor_copy(out=bS[:, :], in_=pb[:, :])

            # ---- T-b + F2 ----
            pbT = pmid.tile([128, 256], BF, tag="mid")
            for q in range(4):
                nc.tensor.transpose(pbT[:, q * 64:(q + 1) * 64],
                                    bS[:, q * 128:(q + 1) * 128],
                                    identW[0:64, 0:64])
            bTS = smp.tile([128, 256], BF, tag="bTS")
            nc.vector.tensor_copy(out=bTS[:, :], in_=pbT[:, :])
            pcst = pmid.tile([128, 32], F32, tag="mid")
            for q in range(4):
                rr = bTS[:, q * 64:(q + 1) * 64].rearrange("p (mk ri) -> p ri mk", ri=2)
                tp = (0, 32 * q)
                nc.tensor.matmul(pcst[32 * q:32 * (q + 1), :], F2rW[:, :],
                                 rr[:, 0:1, :], start=True, stop=False,
                                 tile_position=tp)
                nc.tensor.matmul(pcst[32 * q:32 * (q + 1), :], F2iW[:, :],
                                 rr[:, 1:2, :], start=False, stop=True,
                                 tile_position=tp)
            cstS = smp.tile([128, 32], BF, tag="cstS")
            nc.vector.tensor_copy(out=cstS[:, :], in_=pcst[:, :])

            # ---- mode multiply ----
            py = pmid.tile([128, 32], F32, tag="mid")
            for col in range(32):
                nc.tensor.matmul(py[:, col:col + 1],
                                 mw[:, col * 128:(col + 1) * 128],
                                 cstS[:, col:col + 1], start=True, stop=True)
            yS = smp.tile([128, 32], BF, tag="yS")
            nc.vector.tensor_copy(out=yS[:, :], in_=py[:, :])

            # ---- F3 (inverse d) ----
            pd = pmid.tile([128, 256], F32, tag="mid")
            for g in range(8):
                nc.tensor.matmul(pd[0:64, g * 32:(g + 1) * 32],
                                 F3rW[:, g * 64:(g + 1) * 64], yS[:, :],
                                 start=True, stop=True)
                nc.tensor.matmul(pd[64:128, g * 32:(g + 1) * 32],
                                 F3iW[:, g * 64:(g + 1) * 64], yS[:, :],
                                 start=True, stop=True, tile_position=(0, 64))
            dS = smp.tile([128, 256], BF, tag="dS")
            nc.vector.tensor_copy(out=dS[:, :], in_=pd[:, :])

            # ---- T-d ----
            pdT = pmid.tile([64, 512], BF, tag="mid")
            for ri in range(2):
                for g in range(8):
                    nc.tensor.transpose(
                        pdT[ri * 32:(ri + 1) * 32, g * 64:(g + 1) * 64],
                        dS[ri * 64:(ri + 1) * 64, g * 32:(g + 1) * 32],
                        identW[ri * 64:(ri + 1) * 64, ri * 64:(ri + 1) * 64],
                        tile_position=(ri * 64, ri * 32))
            dTS = smp.tile([64, 512], BF, tag="dTS")
            nc.vector.tensor_copy(
                out=dTS[:, :].rearrange("p (g ds o) -> p g o ds", g=8, ds=8, o=8),
                in_=pdT[:, :].rearrange("p (g o ds) -> p g o ds", g=8, ds=8, o=8))

            # ---- F4' + conv + gelu ----
            for dh in range(4):
                for cc in range(4):
                    px = pX.tile([128, 1024], F32, tag="px")
                    for s in range(2):
                        ck = cc * 1024 + s * 512
                        nc.tensor.matmul(px[:, s * 512:(s + 1) * 512],
                                         dTS[:, dh * 128:(dh + 1) * 128],
                                         T4W[:, ck:ck + 512],
                                         start=True, stop=False)
                        nc.tensor.matmul(px[:, s * 512:(s + 1) * 512],
                                         convW[:, li * 128:(li + 1) * 128],
                                         xcur[:, dh * 4096 + ck: dh * 4096 + ck + 512],
                                         start=False, stop=True)
                    off = dh * 4096 + cc * 1024
                    if li < 3:
                        nc.scalar.activation(xnext[:, off:off + 1024], px[:, :],
                                             GELU, bias=biasT[:, 1 + li:2 + li],
                                             scale=1.0)
                    else:
                        nc.vector.tensor_scalar_add(xnext[:, off:off + 1024],
                                                    px[:, :], biasT[:, 1 + li:2 + li])
            xcur, xnext = xnext, xcur

        # ---- fc1 / fc2 / hard / out ----
        from concourse import mybir as _mb
        for pp in range(4):
            for dh in range(4):
                outS = op.tile([128, 4096], BF, tag="outS")
                for cc in range(4):
                    pz = pX.tile([128, 1024], F32, tag="px")
                    for s in range(2):
                        ck = dh * 4096 + cc * 1024 + s * 512
                        nc.tensor.matmul(pz[:, s * 512:(s + 1) * 512],
                                         fc1W[:, pp * 128:(pp + 1) * 128],
                                         xcur[:, ck:ck + 512],
                                         start=True, stop=True)
                    zS = zp.tile([128, 1024], BF, tag="zS")
                    nc.scalar.activation(zS[:, :], pz[:, :], GELU,
                                         bias=biasT[:, 5:6], scale=1.0)
                    pf = pX.tile([128, 1024], F32, tag="px")
                    for s in range(2):
                        ck = dh * 4096 + cc * 1024 + s * 512
                        nc.tensor.matmul(pf[:, s * 512:(s + 1) * 512], fc2W[:, :],
                                         zS[:, s * 512:(s + 1) * 512],
                                         start=True, stop=False)
                        nc.tensor.matmul(pf[:, s * 512:(s + 1) * 512],
                                         hardW[:, pp * 128:(pp + 1) * 128],
                                         uT[:, ck:ck + 512],
                                         start=False, stop=True)
                    nc.vector.tensor_scalar_add(outS[:, cc * 1024:(cc + 1) * 1024],
                                                pf[:, :], biasT[:, 6:7])
                for dlr in range(4):
                    d = dh * 16 + 4 * pp + dlr
                    nc.sync.dma_start(out=out_d[0:6, d, :, :],
                                      in_=outS[dlr * 32:dlr * 32 + 6, :])
    nc.finalize()
    return nc


# ---------------------------------------------------------------------------
# compile + run (pjrt via axon), cached per weight set
# ---------------------------------------------------------------------------

_COMPILED = {}


def _make_runner(nc):
    import jax
    from jax.sharding import Mesh, PartitionSpec
    try:
        from jax.experimental.shard_map import shard_map
    except Exception:
        from jax.shard_map import shard_map
    from concourse import mybir
    from concourse import bass2jax

    bass2jax.install_neuronx_cc_hook()
    n_cores = 8

    partition_name = (nc.partition_id_tensor.name
                      if nc.partition_id_tensor else None)
    in_names, out_names, out_avals, zero_outs = [], [], [], []
    for alloc in nc.m.functions[0].allocations:
        if not isinstance(alloc, mybir.MemoryLocationSet):
            continue
        if not alloc.memorylocations:
            continue
        name = alloc.memorylocations[0].name
        if alloc.kind == "ExternalInput":
            if name != partition_name:
                in_names.append(name)
        elif alloc.kind == "ExternalOutput":
            shape = tuple(alloc.tensor_shape)
            dtype = mybir.dt.np(alloc.dtype)
            out_names.append(name)
            out_avals.append(jax.core.ShapedArray(shape, dtype))
            zero_outs.append((shape, dtype))
    n_params = len(in_names)
    n_outs = len(out_names)
    all_names = in_names + out_names
    if partition_name is not None:
        all_names = all_names + [partition_name]

    def _body(*args):
        operands = list(args)
        if partition_name is not None:
            operands.append(bass2jax.partition_id_tensor())
        outs = bass2jax._bass_exec_p.bind(
            *operands,
            out_avals=tuple(out_avals),
            in_names=tuple(all_names),
            out_names=tuple(out_names),
            lowering_input_output_aliases=(),
            sim_require_finite=True,
            sim_require_nnan=True,
            nc=nc,
        )
        return tuple(outs)

    devices = jax.devices()[:n_cores]
    mesh = Mesh(np.asarray(devices), ("core",))
    in_specs = (PartitionSpec("core"),) * (n_params + n_outs)
    out_specs = (PartitionSpec("core"),) * n_outs
    donate = tuple(range(n_params, n_params + n_outs))
    sharded = jax.jit(
        shard_map(_body, mesh=mesh, in_specs=in_specs, out_specs=out_specs,
                  check_rep=False),
        donate_argnums=donate, keep_unused=True)

    def run(u_batch):
        # u_batch [8, 3, 64, 64, 64] fp32 -> [8, 6, 64, 64, 64] (bf16 np)
        concat_in = [np.concatenate([u_batch[c] for c in range(n_cores)], axis=0)]
        concat_zeros = [np.zeros((n_cores * s[0],) + s[1:], d)
                        for (s, d) in zero_outs]
        out_arrs = sharded(*concat_in, *concat_zeros)
        o = np.asarray(out_arrs[0]).reshape(n_cores, *zero_outs[0][0])
        return o

    return run


def _get_compiled(p):
    key = _wb_key(p)
    if key not in _COMPILED:
        nc = _build_nc(p)
        _COMPILED[key] = (_make_runner(nc), nc)
    return _COMPILED[key]


def _prep_u(u):
    # [B, 3, 64, 64, 64] f32 -> [B, 48, 16384] bf16 in the device layout
    B = u.shape[0]
    v = u.reshape(B, 3, 4, 16, 64, 64).transpose(0, 1, 3, 2, 4, 5)
    return np.ascontiguousarray(v.reshape(B, 48, 16384).astype(BF16NP))


def kernel(**inputs):
    u = np.asarray(inputs["u"], np.float32)
    p = _make_weights(inputs)
    run, _ = _get_compiled(p)
    out = run(_prep_u(u))
    return out.astype(np.float32)


def profile_kernel(**inputs):
    """Run once with NTFF tracing; returns (out, exec_time_ns)."""
    global LAST_EXEC_NS
    from concourse import bass_utils
    u = np.asarray(inputs["u"], np.float32)
    p = _make_weights(inputs)
    _, nc = _get_compiled(p)
    up = _prep_u(u)
    in_maps = [{"u": up[i]} for i in range(8)]
    res = bass_utils.run_bass_kernel_spmd(nc, in_maps, list(range(8)), trace=True)
    LAST_EXEC_NS = res.exec_time_ns
    out = np.stack([np.asarray(r["out"]) for r in res.results])
    return out.astype(np.float32), res.exec_time_ns


# revision 21
# speedup vs baseline: 2622.9565x; 1154.8142x over previous
"""FNO3d (RCLN v3) Bass/Tile kernel for 8 NeuronCores (trn2).

Data-parallel: 1 batch sample per core, no collectives. The spectral conv
keeps 4 modes/axis, implemented as small dense real-DFT matmuls on the PE
with block-diagonal weight packing so the 128-lane contraction dim is full.

Per-core pipeline (all activations bf16 in SBUF, fp32 PSUM):
  lift (3->8) -> 4x [ PE-transpose to spatial layout -> fused (h,w)-DFT ->
  d-DFT -> per-mode complex mix -> inverse d -> fused inverse (h,w) matmul
  accumulated with the 1x1 conv in PSUM -> gelu drain ] -> fc1+gelu -> fc2
  (+0.3 scale folded into fc2 weights) -> out. Hard path u*nu is added via
  an SBUF dma-accum into the output staging tile.

Weights are baked into the NEFF as inline consts; only `u` ships per call.
"""
import os
import sys
import functools
import hashlib

os.environ.setdefault("JAX_PLATFORMS", "axon,cpu")

import numpy as np
import ml_dtypes

BF16NP = ml_dtypes.bfloat16
N = 64
M = 4
KD = np.array([0, 1, 2, 3, 60, 61, 62, 63])
LAM = 0.3

LAST_EXEC_NS = None


# ---------------------------------------------------------------------------
# host-side weight building (validated in dataflow_check.py)
# ---------------------------------------------------------------------------

def _dft_mats():
    n = np.arange(N)
    FW = np.exp(-2j * np.pi * np.outer(n, np.arange(M)) / N)      # [64, 4]
    FH = np.exp(-2j * np.pi * np.outer(n, KD) / N)                # [64, 8]
    GD = np.exp(2j * np.pi * np.outer(KD, n) / N) / N             # [8, 64]
    IWr = np.zeros((M, N)); IWi = np.zeros((M, N))
    for k in range(M):
        e = np.zeros(N // 2 + 1, complex); e[k] = 1.0
        IWr[k] = np.fft.irfft(e, n=N)
        e = np.zeros(N // 2 + 1, complex); e[k] = 1j
        IWi[k] = np.fft.irfft(e, n=N)
    return FW, FH, GD, IWr, IWi


def _assemble_specw(layer):
    w1, w2, w3, w4 = [np.asarray(w) for w in layer]
    W = np.zeros((8, 8, 8, 8, M), np.complex64)
    W[:, :, :4, :4, :] = w1
    W[:, :, 4:, :4, :] = w2
    W[:, :, :4, 4:, :] = w3
    W[:, :, 4:, 4:, :] = w4
    return W


def _make_weights(inputs):
    FW, FH, GD, IWr, IWi = _dft_mats()
    FHr, FHi = FH.real, FH.imag
    GDr, GDi = GD.real, GD.imag
    p = {}

    fc0_w = np.asarray(inputs["fc0_w"], np.float32)
    L = np.zeros((48, 128), np.float32)
    for c in range(3):
        for dl in range(16):
            for o in range(8):
                L[c * 16 + dl, dl * 8 + o] = fc0_w[o, c]
    p["lift"] = L

    Mf = np.zeros((32, 128, 64), np.float32)
    for hq in range(32):
        for hp in range(2):
            h = 2 * hq + hp
            for w in range(N):
                blk = np.einsum("m,k->mk", FH[h], FW[w])
                Mf[hq, hp * 64 + w, 0::2] = blk.real.reshape(-1)
                Mf[hq, hp * 64 + w, 1::2] = blk.imag.reshape(-1)
    # device layout: [128, 32*64], chunk hq at cols [hq*64:(hq+1)*64]
    p["Mf"] = np.concatenate([Mf[hq] for hq in range(32)], axis=1)

    F2r = np.zeros((128, 32), np.float32)
    F2i = np.zeros((128, 32), np.float32)
    for cc in range(2):
        for d in range(N):
            for nn in range(8):
                F2r[cc * 64 + d, cc * 16 + nn * 2 + 0] = FHr[d, nn]
                F2r[cc * 64 + d, cc * 16 + nn * 2 + 1] = FHi[d, nn]
                F2i[cc * 64 + d, cc * 16 + nn * 2 + 0] = -FHi[d, nn]
                F2i[cc * 64 + d, cc * 16 + nn * 2 + 1] = FHr[d, nn]
    p["F2r"], p["F2i"] = F2r, F2i

    modes = []
    for li in range(4):
        W = _assemble_specw(inputs["spec_ws"][li])   # [c,o,n,mi,k]
        mats = np.zeros((32, 128, 128), np.float32)
        for mi in range(8):
            for k in range(M):
                col = mi * 4 + k
                Wr = W[:, :, :, mi, k].real
                Wi = W[:, :, :, mi, k].imag
                for nn in range(8):
                    r0 = nn * 2
                    c0 = nn * 2
                    for c in range(8):
                        for o in range(8):
                            mats[col, c * 16 + r0 + 0, o * 16 + c0 + 0] = Wr[c, o, nn]
                            mats[col, c * 16 + r0 + 0, o * 16 + c0 + 1] = Wi[c, o, nn]
                            mats[col, c * 16 + r0 + 1, o * 16 + c0 + 0] = -Wi[c, o, nn]
                            mats[col, c * 16 + r0 + 1, o * 16 + c0 + 1] = Wr[c, o, nn]
        modes.append(np.concatenate([mats[col] for col in range(32)], axis=1))
    p["modes"] = modes    # [4][128, 4096]

    F3r = np.zeros((128, 512), np.float32)
    F3i = np.zeros((128, 512), np.float32)
    for g in range(8):
        for o in range(8):
            for nn in range(8):
                for ds in range(8):
                    d = g * 8 + ds
                    cix = g * 64 + o * 8 + ds
                    F3r[o * 16 + nn * 2 + 0, cix] = GDr[nn, d]
                    F3r[o * 16 + nn * 2 + 1, cix] = -GDi[nn, d]
                    F3i[o * 16 + nn * 2 + 0, cix] = GDi[nn, d]
                    F3i[o * 16 + nn * 2 + 1, cix] = GDr[nn, d]
    p["F3r"], p["F3i"] = F3r, F3i

    T4 = np.zeros((64, 4096), np.float32)
    for mi in range(8):
        for k in range(M):
            Tr = np.outer(GDr[mi], IWr[k]) + np.outer(GDi[mi], IWi[k])
            Ti = np.outer(GDr[mi], IWi[k]) - np.outer(GDi[mi], IWr[k])
            T4[0 * 32 + mi * 4 + k] = Tr.reshape(-1)
            T4[1 * 32 + mi * 4 + k] = Ti.reshape(-1)
    p["T4"] = T4

    convs = np.zeros((128, 512), np.float32)
    for li in range(4):
        cw = np.asarray(inputs["conv_ws"][li], np.float32)
        for dl in range(16):
            convs[dl * 8:(dl + 1) * 8, li * 128 + dl * 8: li * 128 + (dl + 1) * 8] = cw.T
    p["convs"] = convs

    fc1_w = np.asarray(inputs["fc1_w"], np.float32)
    fc1s = np.zeros((128, 512), np.float32)
    for pp in range(4):
        for dlr in range(4):
            dl = 4 * pp + dlr
            fc1s[dl * 8:(dl + 1) * 8, pp * 128 + dlr * 32: pp * 128 + (dlr + 1) * 32] = fc1_w.T
    p["fc1s"] = fc1s

    fc2_w = np.asarray(inputs["fc2_w"], np.float32)
    F2m = np.zeros((128, 128), np.float32)
    for dlr in range(4):
        F2m[dlr * 32:(dlr + 1) * 32, dlr * 32:dlr * 32 + 6] = LAM * fc2_w.T
    p["fc2"] = F2m

    # hard path: out[dlr*32+c] += nu*u[c, dl=4pp+dlr] via fc2-PSUM accumulation
    nu = float(np.asarray(inputs["nu"]))
    Hm = np.zeros((48, 512), np.float32)
    for pp in range(4):
        for dlr in range(4):
            dl = 4 * pp + dlr
            for c in range(3):
                Hm[c * 16 + dl, pp * 128 + dlr * 32 + c] = nu
    p["hard"] = Hm

    # bias table [128, 8] fp32: col0 lift, col1-4 conv li, col5 fc1, col6 fc2
    B = np.zeros((128, 8), np.float32)
    B[:, 0] = np.tile(np.asarray(inputs["fc0_b"], np.float32), 16)
    for li in range(4):
        B[:, 1 + li] = np.tile(np.asarray(inputs["conv_bs"][li], np.float32), 16)
    B[:, 5] = np.tile(np.asarray(inputs["fc1_b"], np.float32), 4)
    fb = np.zeros(32, np.float32)
    fb[:6] = LAM * np.asarray(inputs["fc2_b"], np.float32)
    B[:, 6] = np.tile(fb, 4)
    p["bias"] = B

    return p


def _wb_key(p):
    h = hashlib.sha256()
    for k in sorted(p.keys()):
        v = p[k]
        if isinstance(v, list):
            for a in v:
                h.update(np.ascontiguousarray(a).tobytes())
        elif isinstance(v, np.ndarray):
            h.update(np.ascontiguousarray(v).tobytes())
        else:
            h.update(repr(v).encode())
    return h.hexdigest()


# ---------------------------------------------------------------------------
# bass kernel
# ---------------------------------------------------------------------------

def _build_nc(p):
    import concourse.bass as bass
    import concourse.tile as tile
    from concourse import bacc, mybir
    from contextlib import ExitStack

    BF = mybir.dt.bfloat16
    F32 = mybir.dt.float32
    GELU = mybir.ActivationFunctionType.Gelu_apprx_tanh
    bf = lambda a: np.ascontiguousarray(np.asarray(a, np.float32).astype(BF16NP))

    nc = bacc.Bacc("TRN2", target_bir_lowering=False, debug=False, num_devices=8)
    # u arrives host-prearranged: [48, 16384] bf16, p=c*16+dl, f=dh*4096+h*64+w
    u_d = nc.declare_dram_parameter("u", [48, 16384], BF, isOutput=False)
    out_d = nc.declare_dram_parameter("out", [6, 64, 64, 64], BF, isOutput=True)

    identD = nc.inline_tensor(np.eye(128, dtype=BF16NP), name="identc")
    liftD = nc.inline_tensor(bf(p["lift"]), name="liftc")
    MfD = nc.inline_tensor(bf(p["Mf"]), name="mfc")
    F2rD = nc.inline_tensor(bf(p["F2r"]), name="f2rc")
    F2iD = nc.inline_tensor(bf(p["F2i"]), name="f2ic")
    modeD = [nc.inline_tensor(bf(p["modes"][li]), name=f"modec{li}") for li in range(4)]
    F3rD = nc.inline_tensor(bf(p["F3r"]), name="f3rc")
    F3iD = nc.inline_tensor(bf(p["F3i"]), name="f3ic")
    T4D = nc.inline_tensor(bf(p["T4"]), name="t4c")
    convD = nc.inline_tensor(bf(p["convs"]), name="convc")
    fc1D = nc.inline_tensor(bf(p["fc1s"]), name="fc1c")
    fc2D = nc.inline_tensor(bf(p["fc2"]), name="fc2c")
    hardD = nc.inline_tensor(bf(p["hard"]), name="hardc")
    biasD = nc.inline_tensor(np.asarray(p["bias"], np.float32), name="biasc")

    with tile.TileContext(nc) as tc, ExitStack() as ctx:
        cpool = ctx.enter_context(tc.tile_pool(name="const", bufs=1))
        mwp = ctx.enter_context(tc.tile_pool(name="mw", bufs=2))
        xsp = ctx.enter_context(tc.tile_pool(name="xsp", bufs=4))
        smp = ctx.enter_context(tc.tile_pool(name="small", bufs=2))
        zp = ctx.enter_context(tc.tile_pool(name="zp", bufs=2))
        op = ctx.enter_context(tc.tile_pool(name="outp", bufs=2))
        pmid = ctx.enter_context(tc.tile_pool(name="pmid", bufs=3, space="PSUM"))
        pX = ctx.enter_context(tc.tile_pool(name="px", bufs=2, space="PSUM"))

        identW = cpool.tile([128, 128], BF)
        liftW = cpool.tile([48, 128], BF)
        MfW = cpool.tile([128, 2048], BF)
        F2rW = cpool.tile([128, 32], BF)
        F2iW = cpool.tile([128, 32], BF)
        F3rW = cpool.tile([128, 512], BF)
        F3iW = cpool.tile([128, 512], BF)
        T4W = cpool.tile([64, 4096], BF)
        convW = cpool.tile([128, 512], BF)
        fc1W = cpool.tile([128, 512], BF)
        fc2W = cpool.tile([128, 128], BF)
        hardW = cpool.tile([48, 512], BF)
        biasT = cpool.tile([128, 8], F32)
        uT = cpool.tile([48, 16384], BF)
        xA = cpool.tile([128, 16384], BF)
        xB = cpool.tile([128, 16384], BF)

        for t, d in [(identW, identD), (liftW, liftD), (MfW, MfD), (F2rW, F2rD),
                     (F2iW, F2iD), (F3rW, F3rD), (F3iW, F3iD), (T4W, T4D),
                     (convW, convD), (fc1W, fc1D), (fc2W, fc2D),
                     (hardW, hardD), (biasT, biasD)]:
            nc.sync.dma_start(out=t[:, :], in_=d[:, :])

        nc.sync.dma_start(out=uT[:, :], in_=u_d[:, :])

        # ---- lift ----
        for ch in range(16):
            px = pX.tile([128, 1024], F32, tag="px")
            for s in range(2):
                nc.tensor.matmul(px[:, s * 512:(s + 1) * 512], liftW[:, :],
                                 uT[:, ch * 1024 + s * 512: ch * 1024 + (s + 1) * 512],
                                 start=True, stop=True)
            nc.vector.tensor_scalar_add(xA[:, ch * 1024:(ch + 1) * 1024],
                                        px[:, :], biasT[:, 0:1])


        xcur, xnext = xA, xB
        for li in range(4):
            mw = mwp.tile([128, 4096], BF, tag="mw")
            nc.sync.dma_start(out=mw[:, :], in_=modeD[li][:, :])

            # ---- Tx + F1 ----
            pb = pmid.tile([64, 512], F32, tag="mid")
            for hq in range(32):
                ptx = pmid.tile([128, 512], BF, tag="mid")
                for dh in range(4):
                    nc.tensor.transpose(
                        ptx[:, dh * 128:(dh + 1) * 128],
                        xcur[:, dh * 4096 + hq * 128: dh * 4096 + (hq + 1) * 128],
                        identW[:, :])
                xs = xsp.tile([128, 512], BF, tag="xs")
                nc.vector.tensor_copy(
                    out=xs[:, :].rearrange("p (c dh dl) -> p dh dl c",
                                            c=8, dh=4, dl=16),
                    in_=ptx[:, :].rearrange("p (dh dl c) -> p dh dl c",
                                            c=8, dh=4, dl=16))
                nc.tensor.matmul(pb[:, :], MfW[:, hq * 64:(hq + 1) * 64], xs[:, :],
                                 start=(hq == 0), stop=(hq == 31))
            bS = smp.tile([64, 512], BF, tag="bS")
            nc.vector.tensor_copy(out=bS[:, :], in_=pb[:, :])

            # ---- T-b + F2 ----
            pbT = pmid.tile([128, 256], BF, tag="mid")
            for q in range(4):
                nc.tensor.transpose(pbT[:, q * 64:(q + 1) * 64],
                                    bS[:, q * 128:(q + 1) * 128],
                                    identW[0:64, 0:64])
            bTS = smp.tile([128, 256], BF, tag="bTS")
            nc.vector.tensor_copy(out=bTS[:, :], in_=pbT[:, :])
            pcst = pmid.tile([128, 32], F32, tag="mid")
            for q in range(4):
                rr = bTS[:, q * 64:(q + 1) * 64].rearrange("p (mk ri) -> p ri mk", ri=2)
                tp = (0, 32 * q)
                nc.tensor.matmul(pcst[32 * q:32 * (q + 1), :], F2rW[:, :],
                                 rr[:, 0:1, :], start=True, stop=False,
                                 tile_position=tp)
                nc.tensor.matmul(pcst[32 * q:32 * (q + 1), :], F2iW[:, :],
                                 rr[:, 1:2, :], start=False, stop=True,
                                 tile_position=tp)
            cstS = smp.tile([128, 32], BF, tag="cstS")
            nc.vector.tensor_copy(out=cstS[:, :], in_=pcst[:, :])

            # ---- mode multiply ----
            py = pmid.tile([128, 32], F32, tag="mid")
            for col in range(32):
                nc.tensor.matmul(py[:, col:col + 1],
                                 mw[:, col * 128:(col + 1) * 128],
                                 cstS[:, col:col + 1], start=True, stop=True)
            yS = smp.tile([128, 32], BF, tag="yS")
            nc.vector.tensor_copy(out=yS[:, :], in_=py[:, :])

            # ---- F3 (inverse d) ----
            pd = pmid.tile([128, 256], F32, tag="mid")
            for g in range(8):
                nc.tensor.matmul(pd[0:64, g * 32:(g + 1) * 32],
                                 F3rW[:, g * 64:(g + 1) * 64], yS[:, :],
                                 start=True, stop=True)
                nc.tensor.matmul(pd[64:128, g * 32:(g + 1) * 32],
                                 F3iW[:, g * 64:(g + 1) * 64], yS[:, :],
                                 start=True, stop=True, tile_position=(0, 64))
            dS = smp.tile([128, 256], BF, tag="dS")
            nc.vector.tensor_copy(out=dS[:, :], in_=pd[:, :])

            # ---- T-d ----
            pdT = pmid.tile([64, 512], BF, tag="mid")
            for ri in range(2):
                for g in range(8):
                    nc.tensor.transpose(
                        pdT[ri * 32:(ri + 1) * 32, g * 64:(g + 1) * 64],
                        dS[ri * 64:(ri + 1) * 64, g * 32:(g + 1) * 32],
                        identW[ri * 64:(ri + 1) * 64, ri * 64:(ri + 1) * 64],
                        tile_position=(ri * 64, ri * 32))
            dTS = smp.tile([64, 512], BF, tag="dTS")
            nc.vector.tensor_copy(
                out=dTS[:, :].rearrange("p (g ds o) -> p g o ds", g=8, ds=8, o=8),
                in_=pdT[:, :].rearrange("p (g o ds) -> p g o ds", g=8, ds=8, o=8))

            # ---- F4' + conv + gelu ----
            for dh in range(4):
                for cc in range(4):
                    px = pX.tile([128, 1024], F32, tag="px")
                    for s in range(2):
                        ck = cc * 1024 + s * 512
                        nc.tensor.matmul(px[:, s * 512:(s + 1) * 512],
                                         dTS[:, dh * 128:(dh + 1) * 128],
                                         T4W[:, ck:ck + 512],
                                         start=True, stop=False)
                        nc.tensor.matmul(px[:, s * 512:(s + 1) * 512],
                                         convW[:, li * 128:(li + 1) * 128],
                                         xcur[:, dh * 4096 + ck: dh * 4096 + ck + 512],
                                         start=False, stop=True)
                    off = dh * 4096 + cc * 1024
                    if li < 3:
                        nc.scalar.activation(xnext[:, off:off + 1024], px[:, :],
                                             GELU, bias=biasT[:, 1 + li:2 + li],
                                             scale=1.0)
                    else:
                        nc.vector.tensor_scalar_add(xnext[:, off:off + 1024],
                                                    px[:, :], biasT[:, 1 + li:2 + li])
            xcur, xnext = xnext, xcur

        # ---- fc1 / fc2 / hard / out ----
        from concourse import mybir as _mb
        for pp in range(4):
            for dh in range(4):
                outS = op.tile([128, 4096], BF, tag="outS")
                for cc in range(4):
                    pz = pX.tile([128, 1024], F32, tag="px")
                    for s in range(2):
                        ck = dh * 4096 + cc * 1024 + s * 512
                        nc.tensor.matmul(pz[:, s * 512:(s + 1) * 512],
                                         fc1W[:, pp * 128:(pp + 1) * 128],
                                         xcur[:, ck:ck + 512],
                                         start=True, stop=True)
                    zS = zp.tile([128, 1024], BF, tag="zS")
                    nc.scalar.activation(zS[:, :], pz[:, :], GELU,
                                         bias=biasT[:, 5:6], scale=1.0)
                    pf = pX.tile([128, 1024], F32, tag="px")
                    for s in range(2):
                        ck = dh * 4096 + cc * 1024 + s * 512
                        nc.tensor.matmul(pf[:, s * 512:(s + 1) * 512], fc2W[:, :],
                                         zS[:, s * 512:(s + 1) * 512],
                                         start=True, stop=False)
                        nc.tensor.matmul(pf[:, s * 512:(s + 1) * 512],
                                         hardW[:, pp * 128:(pp + 1) * 128],
                                         uT[:, ck:ck + 512],
                                         start=False, stop=True)
                    nc.vector.tensor_scalar_add(outS[:, cc * 1024:(cc + 1) * 1024],
                                                pf[:, :], biasT[:, 6:7])
                for dlr in range(4):
                    d = dh * 16 + 4 * pp + dlr
                    nc.sync.dma_start(out=out_d[0:6, d, :, :],
                                      in_=outS[dlr * 32:dlr * 32 + 6, :])
    nc.finalize()
    return nc


# ---------------------------------------------------------------------------
# compile + run (pjrt via axon), cached per weight set
# ---------------------------------------------------------------------------

_COMPILED = {}


def _make_runner(nc):
    import jax
    from jax.sharding import Mesh, PartitionSpec
    try:
        from jax.experimental.shard_map import shard_map
    except Exception:
        from jax.shard_map import shard_map
    from concourse import mybir
    from concourse import bass2jax

    bass2jax.install_neuronx_cc_hook()
    n_cores = 8

    partition_name = (nc.partition_id_tensor.name
                      if nc.partition_id_tensor else None)
    in_names, out_names, out_avals, zero_outs = [], [], [], []
    for alloc in nc.m.functions[0].allocations:
        if not isinstance(alloc, mybir.MemoryLocationSet):
            continue
        if not alloc.memorylocations:
            continue
        name = alloc.memorylocations[0].name
        if alloc.kind == "ExternalInput":
            if name != partition_name:
                in_names.append(name)
        elif alloc.kind == "ExternalOutput":
            shape = tuple(alloc.tensor_shape)
            dtype = mybir.dt.np(alloc.dtype)
            out_names.append(name)
            out_avals.append(jax.core.ShapedArray(shape, dtype))
            zero_outs.append((shape, dtype))
    n_params = len(in_names)
    n_outs = len(out_names)
    all_names = in_names + out_names
    if partition_name is not None:
        all_names = all_names + [partition_name]

    def _body(*args):
        operands = list(args)
        if partition_name is not None:
            operands.append(bass2jax.partition_id_tensor())
        outs = bass2jax._bass_exec_p.bind(
            *operands,
            out_avals=tuple(out_avals),
            in_names=tuple(all_names),
            out_names=tuple(out_names),
            lowering_input_output_aliases=(),
            sim_require_finite=True,
            sim_require_nnan=True,
            nc=nc,
        )
        return tuple(outs)

    devices = jax.devices()[:n_cores]
    mesh = Mesh(np.asarray(devices), ("core",))
    in_specs = (PartitionSpec("core"),) * (n_params + n_outs)
    out_specs = (PartitionSpec("core"),) * n_outs
    donate = tuple(range(n_params, n_params + n_outs))
    sharded = jax.jit(
        shard_map(_body, mesh=mesh, in_specs=in_specs, out_specs=out_specs,
                  check_rep=False),
        donate_argnums=donate, keep_unused=True)

    def run(u_batch):
        # u_batch [8, 3, 64, 64, 64] fp32 -> [8, 6, 64, 64, 64] (bf16 np)
        concat_in = [np.concatenate([u_batch[c] for c in range(n_cores)], axis=0)]
        concat_zeros = [np.zeros((n_cores * s[0],) + s[1:], d)
                        for (s, d) in zero_outs]
        out_arrs = sharded(*concat_in, *concat_zeros)
        o = np.asarray(out_arrs[0]).reshape(n_cores, *zero_outs[0][0])
        return o

    return run


def _get_compiled(p):
    key = _wb_key(p)
    if key not in _COMPILED:
        nc = _build_nc(p)
        _COMPILED[key] = (_make_runner(nc), nc)
    return _COMPILED[key]


def _prep_u(u):
    # [B, 3, 64, 64, 64] f32 -> [B, 48, 16384] bf16 in the device layout
    B = u.shape[0]
    v = u.reshape(B, 3, 4, 16, 64, 64).transpose(0, 1, 3, 2, 4, 5)
    return np.ascontiguousarray(v.reshape(B, 48, 16384).astype(BF16NP))


def kernel(**inputs):
    u = np.asarray(inputs["u"], np.float32)
    p = _make_weights(inputs)
    run, _ = _get_compiled(p)
    out = run(_prep_u(u))
    return out.astype(np.float32)


def _install_ntff_hook():
    """Provide antenv.axon_hooks (absent in this image) so that
    run_bass_kernel_spmd(trace=True) can reach the axon NTFF profiler."""
    import types
    try:
        from antenv.axon_hooks import get_axon_ntff_profile_hook  # noqa
        return
    except ImportError:
        pass
    store = {}
    mod = types.ModuleType("antenv.axon_hooks")
    mod.set_axon_ntff_profile_hook = lambda h: store.__setitem__("h", h)
    mod.get_axon_ntff_profile_hook = lambda: store.get("h")
    sys.modules["antenv.axon_hooks"] = mod
    import antenv
    antenv.axon_hooks = mod
    if "/root/.axon_site" not in sys.path:
        sys.path.insert(0, "/root/.axon_site")
    from trn_agent_boot.trn_boot import _ntff_profile_via_ctypes
    hook = _ntff_profile_via_ctypes("/opt/axon/libaxon_pjrt.so")
    mod.set_axon_ntff_profile_hook(hook)


def profile_kernel(**inputs):
    """Run once with NTFF tracing; returns (out, exec_time_ns)."""
    global LAST_EXEC_NS
    _install_ntff_hook()
    from concourse import bass_utils
    u = np.asarray(inputs["u"], np.float32)
    p = _make_weights(inputs)
    _, nc = _get_compiled(p)
    up = _prep_u(u)
    in_maps = [{"u": up[i]} for i in range(8)]
    res = bass_utils.run_bass_kernel_spmd(nc, in_maps, list(range(8)), trace=True)
    LAST_EXEC_NS = res.exec_time_ns
    out = np.stack([np.asarray(r["out"]) for r in res.results])
    return out.astype(np.float32), res.exec_time_ns
